# revision 9
# baseline (speedup 1.0000x reference)
"""Single-head causal attention on 8 trn2 NeuronCores (one batch element per core).

Problem: x [8, 2048, 1024], Wq/Wk/Wv [1024, 64] -> out [8, 2048, 64]
  q = x@Wq; k = x@Wk; v = x@Wv; out = causal_softmax(q k^T / sqrt(64)) @ v

Strategy (per core, batch-parallel across the 8 cores):
  - Host pre-transposes + pre-tiles x to [P, NSB, EC, SBLK] fp16 so every DMA
    piece is 4KB-contiguous per partition and the QKV projections contract
    over E with E on SBUF partitions.
  - x streams on BOTH HWDGE rings (each ring caps at ~180 GB/s; the 16 DMA
    engines are split between rings), chunk-halves per ring, with each
    block's piece chained on the previous block's completion semaphore —
    rings process queued transfers concurrently, so without the chain every
    piece finishes late together. The first pieces are placed ahead of the
    (NRT-injected) entry barriers; NRT's preamble still gates issue at
    ~6.8us, but nothing of ours adds to it.
  - Q^T/K^T packed as one [Wq|Wk] matmul; partition-swapped copies (K^T at
    rows 0:64, Q^T at rows 64:128) via PE identity-shift matmuls in
    complementary row/col groups (concurrent, ~0.2us/block).
  - Scores transposed (P^T[kv, q]) two tiles at a time in disjoint PE row
    groups; softmax normalization rides the PV matmul via a ones column in V
    (row 64 of PV output = denominator). No max-subtraction needed.
  - exp on ScalarE straight out of PSUM with the 1/sqrt(D) scale folded in;
    ACT table pre-warmed at tile entry. Softmax divide as exp(-ln s) on
    ScalarE (the DVE reciprocal is ~6x slower at this shape).
  - Final block's output is produced in two column halves: columns 0:256 of
    the PV accumulator are final one pair early, so the first half's
    normalize + store overlaps the last pair's exp/PV.
  - Block schedule proj0,proj1,attn0,proj2,attn1,proj3,attn2,attn3 keeps
    independent projection work between attention phases.
"""

import numpy as np

import concourse.bass as bass
import concourse.mybir as mybir
import concourse.tile as tile
from concourse.vector_clock import ScopedClock

S = 2048  # sequence length
E = 1024  # embed dim
D = 64    # head size
B = 8     # batch == number of cores
P = 128   # SBUF partitions
SBLK = 512         # q-block / s-block width (max fp32 matmul moving dim)
EC = E // P        # 8 contraction chunks
NSB = S // SBLK    # 4 s-blocks
NJT = S // P       # 16 kv tiles

f32 = mybir.dt.float32
f16 = mybir.dt.float16
f8 = mybir.dt.float8e4
DR = mybir.MatmulPerfMode.DoubleRow
MMDT = f16          # dtype of all large-matmul operands
MMNP = np.float16   # matching numpy dtype for host-side prep
AF = mybir.ActivationFunctionType

_PATCHED = False


def _patch_tile_drain():
    """The walrus build in this container rejects instructions carrying more
    than one sem wait on the Tile exit Drain. Split the waits across a chain
    of drains, one wait each."""
    global _PATCHED
    if _PATCHED:
        return
    _PATCHED = True

    def _drain_and_barrier(self, tick_clock, wait_clock):
        drain_inst = self.nc.sync.drain()
        wait_clock.add_sem_waits(
            drain_inst.ins, ScopedClock({None: tick_clock.global_clock})
        )
        ins = drain_inst.ins
        si = ins.sync_info
        if si is not None and si.on_wait is not None and len(si.on_wait) > 1:
            waits = list(si.on_wait)
            ins.sync_info = mybir.SyncInfo(
                on_wait=[waits[0]], on_update=list(si.on_update or [])
            )
            for w in waits[1:]:
                d2 = self.nc.sync.drain()
                d2.ins.sync_info = mybir.SyncInfo(on_wait=[w], on_update=[])
        self.nc.all_engine_barrier()
        assert self.sems is not None
        popped = self.nc._tile_sem_poison_stack.pop()
        assert popped is self._sem_poison
        self.nc.clear_and_free_semaphores(list(self.sems.allocated().values()))
        self.nc.all_engine_barrier()

    tile.TileContext._drain_and_barrier = _drain_and_barrier


def _split_multiwaits(nc):
    """This container's walrus rejects instructions carrying more than one
    sem wait (setupSyncWait: 'Too many sync wait commands'). Hoist all but
    the last wait of every instruction onto same-engine NoOps placed
    immediately before it."""
    ctr = 0
    for f in nc.m.functions:
        for bb in f.blocks:
            out = []
            changed = False
            for inst in bb.instructions:
                si = inst.sync_info
                if si is not None and si.on_wait is not None and len(si.on_wait) > 1:
                    waits = list(si.on_wait)
                    for w in waits[:-1]:
                        nop = mybir.InstNoOp(name=f"I-waitsplit-{ctr}")
                        ctr += 1
                        nop.engine = inst.engine
                        nop.sync_info = mybir.SyncInfo(on_wait=[w], on_update=[])
                        out.append(nop)
                    inst.sync_info = mybir.SyncInfo(
                        on_wait=[waits[-1]], on_update=list(si.on_update or [])
                    )
                    changed = True
                out.append(inst)
            if changed:
                bb.instructions = out


def _restructure(nc):
    """Post-build program surgery:
    1. Move the initial input DMAs (weights + x block 0 halves) to the very
       front of the main block so nothing of ours delays their issue (NRT's
       injected preamble still runs first).
    2. Chain x block 1..3 piece DMAs on the previous block's completion
       semaphore per ring, so each ring delivers blocks in order instead of
       round-robin-sharing bandwidth across all queued pieces.
    3. Move the const-pool memsets (Pool engine) from before the entry
       barrier into the tile block.
    4. Delete the main-block entry barrier: cross-engine deps inside the
       tile block are tracked by tile semaphores, and NRT's own preamble
       barrier already synchronizes engine start."""
    f = nc.m.functions[0]
    main, tileb = f.blocks[0], f.blocks[1]
    hoist_ids = {id(i) for i in nc._hoist_dma}
    moved = [i for i in tileb.instructions if id(i) in hoist_ids]
    assert len(moved) == len(nc._hoist_dma), (len(moved), len(nc._hoist_dma))
    rest_tile = [i for i in tileb.instructions if id(i) not in hoist_ids]

    for prev, cur in nc._chain_dma:
        upd = prev.sync_info.on_update
        assert upd, f"chain prev {prev.name} has no completion sem"
        sem = upd[0].id
        w = mybir.SyncWait(
            sync_type="semaphore", id=sem, wait_mode="sem-ge-imm", wait_value=16
        )
        si = cur.sync_info
        waits = list(si.on_wait or []) if si is not None else []
        upds = list(si.on_update or []) if si is not None else []
        cur.sync_info = mybir.SyncInfo(on_wait=waits + [w], on_update=upds)

    memsets = [i for i in main.instructions if isinstance(i, mybir.InstMemset)]
    keep = []
    for i in main.instructions:
        if isinstance(i, mybir.InstMemset):
            continue
        if isinstance(i, mybir.InstDrain):
            continue
        if isinstance(i, mybir.InstEventSemaphore) and i.name.startswith("barrier_"):
            continue
        keep.append(i)
    main.instructions = keep[:1] + moved + keep[1:]
    tileb.instructions = memsets + rest_tile


def _attention(ctx, tc, xt, xt8, wqk, wv, wqk8, wv8, y):
    nc = tc.nc
    scale = 1.0 / np.sqrt(D)

    persist = ctx.enter_context(tc.tile_pool(name="persist", bufs=1))
    xpool = ctx.enter_context(tc.tile_pool(name="xts", bufs=1))
    ppool = ctx.enter_context(tc.tile_pool(name="pp", bufs=6))
    rpool = ctx.enter_context(tc.tile_pool(name="rec", bufs=8))
    psproj = ctx.enter_context(tc.tile_pool(name="psproj", bufs=2, space="PSUM"))
    psscore = ctx.enter_context(tc.tile_pool(name="psscore", bufs=2, space="PSUM"))
    pspv = ctx.enter_context(tc.tile_pool(name="pspv", bufs=1, space="PSUM"))
    pstr = ctx.enter_context(tc.tile_pool(name="pstr", bufs=1, space="PSUM"))

    # ---- weights + x stream: both rings, chunk-halves, chained ----------
    EH = EC // 2
    PH = EC // 4  # chunk-pairs per ring half (DoubleRow pairs)
    wqk_sb = persist.tile([P, EC, 2 * D], MMDT, tag="wqk")
    wv_sb = persist.tile([P, EC, D], MMDT, tag="wv")
    wqk8_sb = persist.tile([P, EC // 2, 2, 2 * D], f8, tag="wqk8")
    wv8_sb = persist.tile([P, EC // 2, 2, D], f8, tag="wv8")
    xt_r = xt.rearrange("p (c s) -> p c s", c=EC)
    xt8_r = xt8.rearrange("p (b c o s) -> p b c o s", b=NSB - 1, c=EC // 2, o=2)
    xts0 = xpool.tile([P, EC, SBLK], MMDT, tag="xts0")
    xts8 = []
    for b in range(1, NSB):
        xts8_b = xpool.tile(
            [P, EC // 2, 2, SBLK], f8, tag=f"xts8_{b}", name=f"xts8_{b}"
        )
        xts8.append(xts8_b)
    hoist = [
        nc.sync.dma_start(wqk_sb[:], wqk.rearrange("p (c m) -> p c m", c=EC)),
        nc.sync.dma_start(wv_sb[:], wv.rearrange("p (c m) -> p c m", c=EC)),
        nc.scalar.dma_start(wqk8_sb[:], wqk8.rearrange("p (c o m) -> p c o m", c=EC // 2, o=2)),
        nc.scalar.dma_start(wv8_sb[:], wv8.rearrange("p (c o m) -> p c o m", c=EC // 2, o=2)),
        nc.sync.dma_start(xts0[:, :EH], xt_r[:, :EH]),
        nc.scalar.dma_start(xts0[:, EH:], xt_r[:, EH:]),
    ]
    nc._hoist_dma = [h.ins for h in hoist]
    chain_a = [hoist[4]]
    chain_b = [hoist[5]]
    for b in range(1, NSB):
        chain_a.append(nc.sync.dma_start(xts8[b - 1][:, :PH], xt8_r[:, b - 1, :PH]))
        chain_b.append(nc.scalar.dma_start(xts8[b - 1][:, PH:], xt8_r[:, b - 1, PH:]))
    nc._chain_dma = [
        (c[i].ins, c[i + 1].ins)
        for c in (chain_a, chain_b)
        for i in range(NSB - 1)
    ]
    EORD = [4, 5, 6, 7, 0, 1, 2, 3]  # scalar-ring half lands first
    PORD = [2, 3, 0, 1]              # ditto, in DoubleRow chunk-pairs

    # ---- PE warm-up: ramp the PE clock while x block 0 streams in --------
    warm_in = persist.tile([P, SBLK], MMDT, tag="warm")
    nc.vector.memset(warm_in[:], 0.25)
    wt = pstr.tile([P, SBLK], f32, tag="tr")
    for _ in range(3):
        nc.tensor.matmul(wt[:], warm_in[:, :P], warm_in[:], start=True, stop=True)

    # ---- constants -------------------------------------------------------
    ident = persist.tile([P, P], f32, tag="ident")
    nc.gpsimd.memset(ident[:], 0.0)
    nc.gpsimd.affine_select(
        out=ident[:], in_=ident[:],
        compare_op=mybir.AluOpType.not_equal, fill=1.0,
        base=0, pattern=[[-1, P]], channel_multiplier=1,
    )
    ident16 = persist.tile([P, P], MMDT, tag="ident16")
    nc.vector.tensor_copy(ident16[:], ident[:])

    # causal step mask: maskW[jj, c] = 1 iff c >= jj + SBLK
    maskW = persist.tile([P, 2 * SBLK], f32, tag="maskw")
    nc.gpsimd.memset(maskW[:], 1.0)
    nc.gpsimd.affine_select(
        out=maskW[:], in_=maskW[:],
        compare_op=mybir.AluOpType.is_ge, fill=0.0,
        base=-SBLK, pattern=[[1, 2 * SBLK]], channel_multiplier=-1,
    )
    mask16 = persist.tile([P, 2 * SBLK], MMDT, tag="mask16")
    nc.vector.tensor_copy(mask16[:], maskW[:])

    # pre-warm the ScalarE activation table (Exp/Ln share one table set) so
    # the one-time ~1.3us ACT_TABLE_LOAD is off the first real exp's path
    actwarm = rpool.tile([D, 4], f32, tag="actwarm")
    nc.scalar.activation(actwarm[:], maskW[:D, :4], AF.Exp)

    # ---- persistent activations -----------------------------------------
    # qk: rows 0:64 = Q^T, rows 64:128 = K^T (straight from packed psum)
    qk = persist.tile([P, S], MMDT, tag="qk")
    # partition-swapped copies: K^T at rows 0:64, Q^T at rows 64:128
    kTlo = persist.tile([D, S], MMDT, tag="ktlo")
    qThi = persist.tile([P, S], MMDT, tag="qthi")  # rows 64:128 used
    vT = persist.tile([D, S], MMDT, tag="vt")
    vAug = persist.tile([P, NJT, 2 * D], MMDT, tag="vaug")
    yT = persist.tile([D, S], f32, tag="ytout")
    ones_f32 = persist.tile([P, NJT, D], f32, tag="ones")
    nc.vector.memset(ones_f32[:], 1.0)
    nc.vector.tensor_copy(vAug[:, :, D:], ones_f32[:])

    def proj(b):
        sl = slice(b * SBLK, (b + 1) * SBLK)
        psQK = psproj.tile([P, SBLK], f32, tag="proj")
        if b == 0:
            for i, e in enumerate(EORD):
                nc.tensor.matmul(
                    psQK[:], wqk_sb[:, e, :], xts0[:, e, :],
                    start=(i == 0), stop=(i == EC - 1),
                )
        else:
            for i, pr in enumerate(PORD):
                nc.tensor.matmul(
                    psQK[:], wqk8_sb[:, pr], xts8[b - 1][:, pr],
                    start=(i == 0), stop=(i == len(PORD) - 1), perf_mode=DR,
                )
        nc.vector.tensor_copy(qk[:, sl], psQK[:])
        psV = psproj.tile([P, SBLK], f32, tag="proj")
        if b == 0:
            for i, e in enumerate(EORD):
                nc.tensor.matmul(
                    psV[:D, :], wv_sb[:, e, :], xts0[:, e, :],
                    start=(i == 0), stop=(i == EC - 1),
                )
        else:
            for i, pr in enumerate(PORD):
                nc.tensor.matmul(
                    psV[:D, :], wv8_sb[:, pr], xts8[b - 1][:, pr],
                    start=(i == 0), stop=(i == len(PORD) - 1), perf_mode=DR,
                )
        # partition swap on the PE: identity matmuls in complementary
        # row/col groups run concurrently; emitted after the V matmuls so
        # the qk copy (DVE) has finished by the time they issue.
        pshift = pstr.tile([P, SBLK], f32, tag="tr")
        nc.tensor.matmul(pshift[:D, :], ident16[D:P, D:P], qk[D:P, sl])
        nc.tensor.matmul(pshift[D:P, :], ident16[:D, :D], qk[:D, sl])
        nc.vector.tensor_copy(kTlo[:, sl], pshift[:D, :])
        nc.vector.tensor_copy(qThi[D:P, sl], pshift[D:P, :])
        nc.vector.tensor_copy(vT[:, sl], psV[:D, :])
        for t in range(4):
            j = 4 * b + t
            psv_t = pstr.tile([P, SBLK], MMDT, tag="tr")
            nc.tensor.transpose(
                psv_t[:, :D], vT[:, j * P : (j + 1) * P], ident16[:D, :D]
            )
            nc.vector.tensor_copy(vAug[:, j, :D], psv_t[:, :D])

    def attn(b, tail_cb=None):
        nj = 4 * b + 4
        psO = pspv.tile([P, SBLK], f32, tag="pv")
        pairs = [(jp, jp + 1) for jp in range(0, nj, 2)]

        def scores_pair(pi):
            j0, j1 = pairs[pi]
            ps = psscore.tile([P, 2 * SBLK], f32, tag="score")
            # narrow only the strongly-masked tiles (t>=2); the (0,1) pair
            # stays full-width so one exp can cover both banks contiguously
            o0 = max(0, (j0 - 4 * b) * P)
            o1 = max(0, (j1 - 4 * b) * P)
            o0 = o0 if o0 >= 2 * P else 0
            o1 = o1 if o1 >= 2 * P else 0
            q0 = slice(b * SBLK + o0, (b + 1) * SBLK)
            q1 = slice(b * SBLK + o1, (b + 1) * SBLK)
            # two PE row-groups: rows 0:64 (kTlo/qk) and 64:128 (qk/qThi)
            nc.tensor.matmul(
                ps[:, o0:SBLK], kTlo[:, j0 * P : (j0 + 1) * P], qk[:D, q0],
            )
            nc.tensor.matmul(
                ps[:, SBLK + o1 :], qk[D:P, j1 * P : (j1 + 1) * P], qThi[D:P, q1],
            )
            return (j0, j1, ps)

        inflight = scores_pair(0)
        for pi in range(len(pairs)):
            j0, j1, ps = inflight
            pt = ppool.tile([P, 2 * SBLK], MMDT, tag="pt")
            offs = [max(0, (j - 4 * b) * P) for j in (j0, j1)]
            eoffs = [o if o >= 2 * P else 0 for o in offs]
            if eoffs == [0, 0]:
                # (nearly) fully-visible pair: one batched exp over both banks
                nc.scalar.activation(pt[:], ps[:], AF.Exp, scale=float(scale))
            else:
                # strongly-masked pair: exp only the causally-reachable columns
                for k, off in enumerate(eoffs):
                    nc.scalar.activation(
                        pt[:, k * SBLK + off : (k + 1) * SBLK],
                        ps[:, k * SBLK + off : (k + 1) * SBLK],
                        AF.Exp, scale=float(scale),
                    )
            for k, j in enumerate((j0, j1)):
                t = j - 4 * b
                if t >= 0:
                    off = eoffs[k]
                    nc.vector.tensor_mul(
                        pt[:, k * SBLK + off : (k + 1) * SBLK],
                        pt[:, k * SBLK + off : (k + 1) * SBLK],
                        mask16[:, SBLK - t * P + off : 2 * SBLK - t * P],
                    )
            if pi + 1 < len(pairs):
                inflight = scores_pair(pi + 1)
            if tail_cb is not None and pi == len(pairs) - 1:
                # psO columns untouched by this last pair are already final:
                # normalize + store them while this pair's PV runs
                tail_cb(psO)
            for k, j in enumerate((j0, j1)):
                off = eoffs[k]
                nc.tensor.matmul(
                    psO[:, off:], vAug[:, j, :],
                    pt[:, k * SBLK + off : (k + 1) * SBLK],
                    start=(j == 0), stop=(j == nj - 1),
                )
        return psO

    def out_part(b, psO, c0, c1):
        # rows 64:128 of psO hold the softmax denominator, pre-broadcast.
        sl = slice(b * SBLK + c0, b * SBLK + c1)
        # 1/s as exp(-ln s) on ScalarE: same ACT table as the softmax exp,
        # far faster than the DVE reciprocal at this shape (custom-DVE
        # approx ops don't survive this walrus build)
        rcp = rpool.tile([D, SBLK], f32, tag="rcp")
        lns = rpool.tile([D, SBLK], f32, tag="lns")
        nc.scalar.activation(lns[:, c0:c1], psO[D:P, c0:c1], AF.Ln)
        nc.scalar.activation(rcp[:, c0:c1], lns[:, c0:c1], AF.Exp, scale=-1.0)
        nc.vector.tensor_mul(yT[:, sl], psO[:D, c0:c1], rcp[:, c0:c1])
        eng = nc.scalar if (b == NSB - 1 and c0 > 0) else nc.sync
        eng.dma_start(y[:, sl], yT[:, sl])

    # schedule: keep independent proj work ahead of each attn phase; the
    # last attn phase (exp-latency-bound) overlaps its own output tail
    proj(0)
    proj(1)
    o0 = attn(0)
    out_part(0, o0, 0, SBLK)
    proj(2)
    o1 = attn(1)
    out_part(1, o1, 0, SBLK)
    proj(3)
    o2 = attn(2)
    out_part(2, o2, 0, SBLK)
    half = SBLK // 2
    o3 = attn(3, tail_cb=lambda psO: out_part(3, psO, 0, half))
    out_part(3, o3, half, SBLK)


def build_nc():
    from contextlib import ExitStack

    _patch_tile_drain()
    nc = bass.Bass(target_bir_lowering=False, enable_partition_id=False)
    xt = nc.dram_tensor("xt", [P, EC * SBLK], MMDT, kind="ExternalInput")
    xt8 = nc.dram_tensor("xt8", [P, (NSB - 1) * EC * SBLK], f8, kind="ExternalInput")
    wqk = nc.dram_tensor("wqk", [P, EC * 2 * D], MMDT, kind="ExternalInput")
    wv = nc.dram_tensor("wv", [P, EC * D], MMDT, kind="ExternalInput")
    wqk8 = nc.dram_tensor("wqk8", [P, EC * 2 * D], f8, kind="ExternalInput")
    wv8 = nc.dram_tensor("wv8", [P, EC * D], f8, kind="ExternalInput")
    y = nc.dram_tensor("y", [D, S], f32, kind="ExternalOutput")
    with tile.TileContext(nc) as tc:
        with ExitStack() as ctx:
            _attention(ctx, tc, xt, xt8, wqk, wv, wqk8, wv8, y)
    _restructure(nc)
    return nc


def make_in_maps(x, Wq, Wk, Wv):
    import ml_dtypes

    f8np = ml_dtypes.float8_e4m3fn
    # weights pre-tiled to [P, EC, cols]: row (c p) of W -> [p][c]
    wqk_cat = np.concatenate([Wq, Wk], axis=1).astype(MMNP)  # [E, 2D]
    wqk_arr = np.ascontiguousarray(
        wqk_cat.reshape(EC, P, 2 * D).transpose(1, 0, 2).reshape(P, EC * 2 * D)
    )
    wv_arr = np.ascontiguousarray(
        np.asarray(Wv).astype(MMNP).reshape(EC, P, D).transpose(1, 0, 2).reshape(P, EC * D)
    )
    wqk8_arr = np.ascontiguousarray(wqk_arr.astype(f8np))
    wv8_arr = np.ascontiguousarray(wv_arr.astype(f8np))
    x = np.asarray(x)
    maps = []
    for b in range(B):
        xt = x[b].T.astype(MMNP)  # [E, S]
        # [P, NSB, EC, SBLK]: xa[p, blk, c, s] = xt[c*128+p, blk*512+s]
        xa = xt.reshape(EC, P, NSB, SBLK).transpose(1, 2, 0, 3)
        maps.append(
            {
                "xt": np.ascontiguousarray(xa[:, 0].reshape(P, EC * SBLK)),
                "xt8": np.ascontiguousarray(
                    xa[:, 1:].astype(f8np).reshape(P, (NSB - 1) * EC * SBLK)
                ),
                "wqk": wqk_arr,
                "wv": wv_arr,
                "wqk8": wqk8_arr,
                "wv8": wv8_arr,
            }
        )
    return maps


_NC = None


def kernel(x, Wq, Wk, Wv, _trace=False, _tmpdir=None):
    from concourse.bass_utils import run_bass_kernel_spmd

    global _NC
    if _NC is None:
        _NC = build_nc()
        _split_multiwaits(_NC)  # walrus-only legalization; breaks CoreSim
    in_maps = make_in_maps(x, Wq, Wk, Wv)
    res = run_bass_kernel_spmd(
        _NC, in_maps, core_ids=list(range(B)), trace=_trace, tmpdir=_tmpdir
    )
    out = np.ascontiguousarray(
        np.stack([r["y"].T for r in res.results], axis=0), dtype=np.float32
    )
    if _trace:
        kernel.last_results = res
    return out


# revision 10
# speedup vs baseline: 1.0676x; 1.0676x over previous
"""Single-head causal attention on 8 trn2 NeuronCores (one batch element per core).

Problem: x [8, 2048, 1024], Wq/Wk/Wv [1024, 64] -> out [8, 2048, 64]
  q = x@Wq; k = x@Wk; v = x@Wv; out = causal_softmax(q k^T / sqrt(64)) @ v

Strategy (per core, batch-parallel across the 8 cores):
  - Host pre-transposes + pre-tiles x to [P, NSB, EC, SBLK] fp16 so every DMA
    piece is 4KB-contiguous per partition and the QKV projections contract
    over E with E on SBUF partitions.
  - x streams on BOTH HWDGE rings (each ring caps at ~180 GB/s; the 16 DMA
    engines are split between rings), chunk-halves per ring, with each
    block's piece chained on the previous block's completion semaphore —
    rings process queued transfers concurrently, so without the chain every
    piece finishes late together. The first pieces are placed ahead of the
    (NRT-injected) entry barriers; NRT's preamble still gates issue at
    ~6.8us, but nothing of ours adds to it.
  - Q^T/K^T packed as one [Wq|Wk] matmul; partition-swapped copies (K^T at
    rows 0:64, Q^T at rows 64:128) via PE identity-shift matmuls in
    complementary row/col groups (concurrent, ~0.2us/block).
  - Scores transposed (P^T[kv, q]) two tiles at a time in disjoint PE row
    groups; softmax normalization rides the PV matmul via a ones column in V
    (row 64 of PV output = denominator). No max-subtraction needed.
  - exp on ScalarE straight out of PSUM with the 1/sqrt(D) scale folded in;
    ACT table pre-warmed at tile entry. Softmax divide as exp(-ln s) on
    ScalarE (the DVE reciprocal is ~6x slower at this shape).
  - Final block's output is produced in two column halves: columns 0:256 of
    the PV accumulator are final one pair early, so the first half's
    normalize + store overlaps the last pair's exp/PV.
  - Block schedule proj0,proj1,attn0,proj2,attn1,proj3,attn2,attn3 keeps
    independent projection work between attention phases.
"""

import numpy as np

import concourse.bass as bass
import concourse.mybir as mybir
import concourse.tile as tile
from concourse.vector_clock import ScopedClock

S = 2048  # sequence length
E = 1024  # embed dim
D = 64    # head size
B = 8     # batch == number of cores
P = 128   # SBUF partitions
SBLK = 512         # q-block / s-block width (max fp32 matmul moving dim)
EC = E // P        # 8 contraction chunks
NSB = S // SBLK    # 4 s-blocks
NJT = S // P       # 16 kv tiles

f32 = mybir.dt.float32
f16 = mybir.dt.float16
MMDT = f16          # dtype of all large-matmul operands
MMNP = np.float16   # matching numpy dtype for host-side prep
AF = mybir.ActivationFunctionType

_PATCHED = False


def _patch_tile_drain():
    """The walrus build in this container rejects instructions carrying more
    than one sem wait on the Tile exit Drain. Split the waits across a chain
    of drains, one wait each."""
    global _PATCHED
    if _PATCHED:
        return
    _PATCHED = True

    def _drain_and_barrier(self, tick_clock, wait_clock):
        drain_inst = self.nc.sync.drain()
        wait_clock.add_sem_waits(
            drain_inst.ins, ScopedClock({None: tick_clock.global_clock})
        )
        ins = drain_inst.ins
        si = ins.sync_info
        if si is not None and si.on_wait is not None and len(si.on_wait) > 1:
            waits = list(si.on_wait)
            ins.sync_info = mybir.SyncInfo(
                on_wait=[waits[0]], on_update=list(si.on_update or [])
            )
            for w in waits[1:]:
                d2 = self.nc.sync.drain()
                d2.ins.sync_info = mybir.SyncInfo(on_wait=[w], on_update=[])
        self.nc.all_engine_barrier()
        assert self.sems is not None
        popped = self.nc._tile_sem_poison_stack.pop()
        assert popped is self._sem_poison
        self.nc.clear_and_free_semaphores(list(self.sems.allocated().values()))
        self.nc.all_engine_barrier()

    tile.TileContext._drain_and_barrier = _drain_and_barrier


def _split_multiwaits(nc):
    """This container's walrus rejects instructions carrying more than one
    sem wait (setupSyncWait: 'Too many sync wait commands'). Hoist all but
    the last wait of every instruction onto same-engine NoOps placed
    immediately before it."""
    ctr = 0
    for f in nc.m.functions:
        for bb in f.blocks:
            out = []
            changed = False
            for inst in bb.instructions:
                si = inst.sync_info
                if si is not None and si.on_wait is not None and len(si.on_wait) > 1:
                    waits = list(si.on_wait)
                    for w in waits[:-1]:
                        nop = mybir.InstNoOp(name=f"I-waitsplit-{ctr}")
                        ctr += 1
                        nop.engine = inst.engine
                        nop.sync_info = mybir.SyncInfo(on_wait=[w], on_update=[])
                        out.append(nop)
                    inst.sync_info = mybir.SyncInfo(
                        on_wait=[waits[-1]], on_update=list(si.on_update or [])
                    )
                    changed = True
                out.append(inst)
            if changed:
                bb.instructions = out


def _restructure(nc):
    """Post-build program surgery:
    1. Move the initial input DMAs (weights + x block 0 halves) to the very
       front of the main block so nothing of ours delays their issue (NRT's
       injected preamble still runs first).
    2. Chain x block 1..3 piece DMAs on the previous block's completion
       semaphore per ring, so each ring delivers blocks in order instead of
       round-robin-sharing bandwidth across all queued pieces.
    3. Move the const-pool memsets (Pool engine) from before the entry
       barrier into the tile block.
    4. Delete the main-block entry barrier: cross-engine deps inside the
       tile block are tracked by tile semaphores, and NRT's own preamble
       barrier already synchronizes engine start."""
    f = nc.m.functions[0]
    main, tileb = f.blocks[0], f.blocks[1]
    hoist_ids = {id(i) for i in nc._hoist_dma}
    moved = [i for i in tileb.instructions if id(i) in hoist_ids]
    assert len(moved) == len(nc._hoist_dma), (len(moved), len(nc._hoist_dma))
    rest_tile = [i for i in tileb.instructions if id(i) not in hoist_ids]

    for prev, cur in nc._chain_dma:
        upd = prev.sync_info.on_update
        assert upd, f"chain prev {prev.name} has no completion sem"
        sem = upd[0].id
        w = mybir.SyncWait(
            sync_type="semaphore", id=sem, wait_mode="sem-ge-imm", wait_value=16
        )
        si = cur.sync_info
        waits = list(si.on_wait or []) if si is not None else []
        upds = list(si.on_update or []) if si is not None else []
        cur.sync_info = mybir.SyncInfo(on_wait=waits + [w], on_update=upds)

    memsets = [i for i in main.instructions if isinstance(i, mybir.InstMemset)]
    keep = []
    for i in main.instructions:
        if isinstance(i, mybir.InstMemset):
            continue
        if isinstance(i, mybir.InstDrain):
            continue
        if isinstance(i, mybir.InstEventSemaphore) and i.name.startswith("barrier_"):
            continue
        keep.append(i)
    main.instructions = keep[:1] + moved + keep[1:]
    tileb.instructions = memsets + rest_tile


def _attention(ctx, tc, xt, wqk, wv, y):
    nc = tc.nc
    scale = 1.0 / np.sqrt(D)

    persist = ctx.enter_context(tc.tile_pool(name="persist", bufs=1))
    xpool = ctx.enter_context(tc.tile_pool(name="xts", bufs=1))
    ppool = ctx.enter_context(tc.tile_pool(name="pp", bufs=6))
    rpool = ctx.enter_context(tc.tile_pool(name="rec", bufs=8))
    psproj = ctx.enter_context(tc.tile_pool(name="psproj", bufs=2, space="PSUM"))
    psscore = ctx.enter_context(tc.tile_pool(name="psscore", bufs=2, space="PSUM"))
    pspv = ctx.enter_context(tc.tile_pool(name="pspv", bufs=1, space="PSUM"))
    pstr = ctx.enter_context(tc.tile_pool(name="pstr", bufs=1, space="PSUM"))

    # ---- weights + x stream: both rings, chunk-halves, chained ----------
    EH = EC // 2
    wqk_sb = persist.tile([P, EC, 2 * D], MMDT, tag="wqk")
    wv_sb = persist.tile([P, EC, D], MMDT, tag="wv")
    xt_r = xt.rearrange("p (b c s) -> p b c s", b=NSB, c=EC)
    xts = []
    for b in range(NSB):
        xts_b = xpool.tile([P, EC, SBLK], MMDT, tag=f"xts{b}", name=f"xts_{b}")
        xts.append(xts_b)
    EQ = EC // 4
    hoist = [
        nc.sync.dma_start(wqk_sb[:], wqk.rearrange("p (c m) -> p c m", c=EC)),
        nc.sync.dma_start(wv_sb[:], wv.rearrange("p (c m) -> p c m", c=EC)),
        nc.sync.dma_start(xts[0][:, :EQ], xt_r[:, 0, :EQ]),
        nc.gpsimd.dma_start(xts[0][:, EH:], xt_r[:, 0, EH:]),
    ]
    nc._hoist_dma = [h.ins for h in hoist]
    # sync ring: x0 second quarter then blocks 1-3 first-halves, chained.
    # SWDGE ring (GpSimd-issued, so ScalarE never stalls on chain waits):
    # x0 second half then blocks 1-3 second-halves, chained.
    chain_a = [hoist[2], nc.sync.dma_start(xts[0][:, EQ:EH], xt_r[:, 0, EQ:EH])]
    chain_b = [hoist[3]]
    for b in range(1, NSB):
        chain_a.append(nc.sync.dma_start(xts[b][:, :EH], xt_r[:, b, :EH]))
        chain_b.append(nc.gpsimd.dma_start(xts[b][:, EH:], xt_r[:, b, EH:]))
    nc._chain_dma = [
        (c[i].ins, c[i + 1].ins)
        for c in (chain_a, chain_b)
        for i in range(len(c) - 1)
    ]
    EORD = [0, 1, 2, 3, 4, 5, 6, 7]  # sync-ring quarter lands first

    # ---- PE warm-up: ramp the PE clock while x block 0 streams in --------
    warm_in = persist.tile([P, SBLK], MMDT, tag="warm")
    nc.vector.memset(warm_in[:], 0.25)
    wt = pstr.tile([P, SBLK], f32, tag="tr")
    for _ in range(3):
        nc.tensor.matmul(wt[:], warm_in[:, :P], warm_in[:], start=True, stop=True)

    # ---- constants -------------------------------------------------------
    ident = persist.tile([P, P], f32, tag="ident")
    nc.gpsimd.memset(ident[:], 0.0)
    nc.gpsimd.affine_select(
        out=ident[:], in_=ident[:],
        compare_op=mybir.AluOpType.not_equal, fill=1.0,
        base=0, pattern=[[-1, P]], channel_multiplier=1,
    )
    ident16 = persist.tile([P, P], MMDT, tag="ident16")
    nc.vector.tensor_copy(ident16[:], ident[:])

    # causal step mask: maskW[jj, c] = 1 iff c >= jj + SBLK
    maskW = persist.tile([P, 2 * SBLK], f32, tag="maskw")
    nc.gpsimd.memset(maskW[:], 1.0)
    nc.gpsimd.affine_select(
        out=maskW[:], in_=maskW[:],
        compare_op=mybir.AluOpType.is_ge, fill=0.0,
        base=-SBLK, pattern=[[1, 2 * SBLK]], channel_multiplier=-1,
    )
    mask16 = persist.tile([P, 2 * SBLK], MMDT, tag="mask16")
    nc.vector.tensor_copy(mask16[:], maskW[:])

    # pre-warm the ScalarE activation table (Exp/Ln share one table set) so
    # the one-time ~1.3us ACT_TABLE_LOAD is off the first real exp's path
    actwarm = rpool.tile([D, 4], f32, tag="actwarm")
    nc.scalar.activation(actwarm[:], maskW[:D, :4], AF.Exp)

    # ---- persistent activations -----------------------------------------
    # qk: rows 0:64 = Q^T, rows 64:128 = K^T (straight from packed psum)
    qk = persist.tile([P, S], MMDT, tag="qk")
    # partition-swapped copies: K^T at rows 0:64, Q^T at rows 64:128
    kTlo = persist.tile([D, S], MMDT, tag="ktlo")
    qThi = persist.tile([P, S], MMDT, tag="qthi")  # rows 64:128 used
    vT = persist.tile([D, S], MMDT, tag="vt")
    vAug = persist.tile([P, NJT, 2 * D], MMDT, tag="vaug")
    yT = persist.tile([D, S], f32, tag="ytout")
    ones_f32 = persist.tile([P, NJT, D], f32, tag="ones")
    nc.vector.memset(ones_f32[:], 1.0)
    nc.vector.tensor_copy(vAug[:, :, D:], ones_f32[:])

    def proj(b):
        sl = slice(b * SBLK, (b + 1) * SBLK)
        psQK = psproj.tile([P, SBLK], f32, tag="proj")
        for i, e in enumerate(EORD):
            nc.tensor.matmul(
                psQK[:], wqk_sb[:, e, :], xts[b][:, e, :],
                start=(i == 0), stop=(i == EC - 1),
            )
        nc.vector.tensor_copy(qk[:, sl], psQK[:])
        psV = psproj.tile([P, SBLK], f32, tag="proj")
        for i, e in enumerate(EORD):
            nc.tensor.matmul(
                psV[:D, :], wv_sb[:, e, :], xts[b][:, e, :],
                start=(i == 0), stop=(i == EC - 1),
            )
        # partition swap on the PE: identity matmuls in complementary
        # row/col groups run concurrently; emitted after the V matmuls so
        # the qk copy (DVE) has finished by the time they issue.
        pshift = pstr.tile([P, SBLK], f32, tag="tr")
        nc.tensor.matmul(pshift[:D, :], ident16[D:P, D:P], qk[D:P, sl])
        nc.tensor.matmul(pshift[D:P, :], ident16[:D, :D], qk[:D, sl])
        nc.vector.tensor_copy(kTlo[:, sl], pshift[:D, :])
        nc.vector.tensor_copy(qThi[D:P, sl], pshift[D:P, :])
        nc.vector.tensor_copy(vT[:, sl], psV[:D, :])
        for t in range(4):
            j = 4 * b + t
            psv_t = pstr.tile([P, SBLK], MMDT, tag="tr")
            nc.tensor.transpose(
                psv_t[:, :D], vT[:, j * P : (j + 1) * P], ident16[:D, :D]
            )
            nc.vector.tensor_copy(vAug[:, j, :D], psv_t[:, :D])

    def attn(b, tail_cb=None):
        nj = 4 * b + 4
        psO = pspv.tile([P, SBLK], f32, tag="pv")
        pairs = [(jp, jp + 1) for jp in range(0, nj, 2)]

        def scores_pair(pi):
            j0, j1 = pairs[pi]
            ps = psscore.tile([P, 2 * SBLK], f32, tag="score")
            # narrow only the strongly-masked tiles (t>=2); the (0,1) pair
            # stays full-width so one exp can cover both banks contiguously
            o0 = max(0, (j0 - 4 * b) * P)
            o1 = max(0, (j1 - 4 * b) * P)
            o0 = o0 if o0 >= 2 * P else 0
            o1 = o1 if o1 >= 2 * P else 0
            q0 = slice(b * SBLK + o0, (b + 1) * SBLK)
            q1 = slice(b * SBLK + o1, (b + 1) * SBLK)
            # two PE row-groups: rows 0:64 (kTlo/qk) and 64:128 (qk/qThi)
            nc.tensor.matmul(
                ps[:, o0:SBLK], kTlo[:, j0 * P : (j0 + 1) * P], qk[:D, q0],
            )
            nc.tensor.matmul(
                ps[:, SBLK + o1 :], qk[D:P, j1 * P : (j1 + 1) * P], qThi[D:P, q1],
            )
            return (j0, j1, ps)

        inflight = scores_pair(0)
        for pi in range(len(pairs)):
            j0, j1, ps = inflight
            pt = ppool.tile([P, 2 * SBLK], MMDT, tag="pt")
            offs = [max(0, (j - 4 * b) * P) for j in (j0, j1)]
            eoffs = [o if o >= 2 * P else 0 for o in offs]
            if eoffs == [0, 0]:
                # (nearly) fully-visible pair: one batched exp over both banks
                nc.scalar.activation(pt[:], ps[:], AF.Exp, scale=float(scale))
            else:
                # strongly-masked pair: exp only the causally-reachable columns
                for k, off in enumerate(eoffs):
                    nc.scalar.activation(
                        pt[:, k * SBLK + off : (k + 1) * SBLK],
                        ps[:, k * SBLK + off : (k + 1) * SBLK],
                        AF.Exp, scale=float(scale),
                    )
            for k, j in enumerate((j0, j1)):
                t = j - 4 * b
                if t >= 0:
                    off = eoffs[k]
                    nc.gpsimd.tensor_mul(
                        pt[:, k * SBLK + off : (k + 1) * SBLK],
                        pt[:, k * SBLK + off : (k + 1) * SBLK],
                        mask16[:, SBLK - t * P + off : 2 * SBLK - t * P],
                    )
            if pi + 1 < len(pairs):
                inflight = scores_pair(pi + 1)
            if tail_cb is not None and pi == len(pairs) - 1:
                # psO columns untouched by this last pair are already final:
                # normalize + store them while this pair's PV runs
                tail_cb(psO)
            for k, j in enumerate((j0, j1)):
                off = eoffs[k]
                nc.tensor.matmul(
                    psO[:, off:], vAug[:, j, :],
                    pt[:, k * SBLK + off : (k + 1) * SBLK],
                    start=(j == 0), stop=(j == nj - 1),
                )
        return psO

    def out_part(b, psO, c0, c1):
        # rows 64:128 of psO hold the softmax denominator, pre-broadcast.
        # 1/s as exp(-ln s) on ScalarE: same ACT table as the softmax exp,
        # ~6x faster than the DVE reciprocal at this shape.
        sl = slice(b * SBLK + c0, b * SBLK + c1)
        lns = rpool.tile([D, SBLK], f32, tag="lns")
        nc.scalar.activation(lns[:, c0:c1], psO[D:P, c0:c1], AF.Ln)
        rcp = rpool.tile([D, SBLK], f32, tag="rcp")
        nc.scalar.activation(rcp[:, c0:c1], lns[:, c0:c1], AF.Exp, scale=-1.0)
        nc.vector.tensor_mul(yT[:, sl], psO[:D, c0:c1], rcp[:, c0:c1])
        eng = nc.scalar if (b == NSB - 1 and c0 > 0) else nc.sync
        eng.dma_start(y[:, sl], yT[:, sl])

    # schedule: keep independent proj work ahead of each attn phase; the
    # last attn phase (exp-latency-bound) overlaps its own output tail
    proj(0)
    proj(1)
    o0 = attn(0)
    out_part(0, o0, 0, SBLK)
    proj(2)
    o1 = attn(1)
    out_part(1, o1, 0, SBLK)
    proj(3)
    o2 = attn(2)
    out_part(2, o2, 0, SBLK)
    half = SBLK // 2
    o3 = attn(3, tail_cb=lambda psO: out_part(3, psO, 0, half))
    out_part(3, o3, half, SBLK)


def build_nc():
    from contextlib import ExitStack

    _patch_tile_drain()
    nc = bass.Bass(target_bir_lowering=False, enable_partition_id=False)
    xt = nc.dram_tensor("xt", [P, NSB * EC * SBLK], MMDT, kind="ExternalInput")
    wqk = nc.dram_tensor("wqk", [P, EC * 2 * D], MMDT, kind="ExternalInput")
    wv = nc.dram_tensor("wv", [P, EC * D], MMDT, kind="ExternalInput")
    y = nc.dram_tensor("y", [D, S], f32, kind="ExternalOutput")
    with tile.TileContext(nc) as tc:
        with ExitStack() as ctx:
            _attention(ctx, tc, xt, wqk, wv, y)
    _restructure(nc)
    return nc


def make_in_maps(x, Wq, Wk, Wv):
    # weights pre-tiled to [P, EC, cols]: row (c p) of W -> [p][c]
    wqk_cat = np.concatenate([Wq, Wk], axis=1).astype(MMNP)  # [E, 2D]
    wqk_arr = np.ascontiguousarray(
        wqk_cat.reshape(EC, P, 2 * D).transpose(1, 0, 2).reshape(P, EC * 2 * D)
    )
    wv_arr = np.ascontiguousarray(
        np.asarray(Wv).astype(MMNP).reshape(EC, P, D).transpose(1, 0, 2).reshape(P, EC * D)
    )
    x = np.asarray(x)
    maps = []
    for b in range(B):
        xt = x[b].T.astype(MMNP)  # [E, S]
        # [P, NSB, EC, SBLK]: xa[p, blk, c, s] = xt[c*128+p, blk*512+s]
        xa = xt.reshape(EC, P, NSB, SBLK).transpose(1, 2, 0, 3)
        maps.append(
            {
                "xt": np.ascontiguousarray(xa.reshape(P, NSB * EC * SBLK)),
                "wqk": wqk_arr,
                "wv": wv_arr,
            }
        )
    return maps


_NC = None


def kernel(x, Wq, Wk, Wv, _trace=False, _tmpdir=None):
    from concourse.bass_utils import run_bass_kernel_spmd

    global _NC
    if _NC is None:
        _NC = build_nc()
        _split_multiwaits(_NC)  # walrus-only legalization; breaks CoreSim
    in_maps = make_in_maps(x, Wq, Wk, Wv)
    res = run_bass_kernel_spmd(
        _NC, in_maps, core_ids=list(range(B)), trace=_trace, tmpdir=_tmpdir
    )
    out = np.ascontiguousarray(
        np.stack([r["y"].T for r in res.results], axis=0), dtype=np.float32
    )
    if _trace:
        kernel.last_results = res
    return out


# revision 12
# speedup vs baseline: 1.2880x; 1.2064x over previous
"""Single-head causal attention on 8 trn2 NeuronCores (one batch element per core).

Problem: x [8, 2048, 1024], Wq/Wk/Wv [1024, 64] -> out [8, 2048, 64]
  q = x@Wq; k = x@Wk; v = x@Wv; out = causal_softmax(q k^T / sqrt(64)) @ v

Strategy (per core, batch-parallel across the 8 cores):
  - Host pre-transposes + pre-tiles x to [P, NSB, EC, SBLK] fp16 so every DMA
    piece is 4KB-contiguous per partition and the QKV projections contract
    over E with E on SBUF partitions.
  - x streams on BOTH HWDGE rings (each ring caps at ~180 GB/s; the 16 DMA
    engines are split between rings), chunk-halves per ring, with each
    block's piece chained on the previous block's completion semaphore —
    rings process queued transfers concurrently, so without the chain every
    piece finishes late together. The first pieces are placed ahead of the
    (NRT-injected) entry barriers; NRT's preamble still gates issue at
    ~6.8us, but nothing of ours adds to it.
  - Q^T/K^T packed as one [Wq|Wk] matmul; partition-swapped copies (K^T at
    rows 0:64, Q^T at rows 64:128) via PE identity-shift matmuls in
    complementary row/col groups (concurrent, ~0.2us/block).
  - Scores transposed (P^T[kv, q]) two tiles at a time in disjoint PE row
    groups; softmax normalization rides the PV matmul via a ones column in V
    (row 64 of PV output = denominator). No max-subtraction needed.
  - exp on ScalarE straight out of PSUM with the 1/sqrt(D) scale folded in;
    ACT table pre-warmed at tile entry. Softmax divide as exp(-ln s) on
    ScalarE (the DVE reciprocal is ~6x slower at this shape).
  - Final block's output is produced in two column halves: columns 0:256 of
    the PV accumulator are final one pair early, so the first half's
    normalize + store overlaps the last pair's exp/PV.
  - Block schedule proj0,proj1,attn0,proj2,attn1,proj3,attn2,attn3 keeps
    independent projection work between attention phases.
"""

import numpy as np

import concourse.bass as bass
import concourse.mybir as mybir
import concourse.tile as tile
from concourse.vector_clock import ScopedClock

S = 2048  # sequence length
E = 1024  # embed dim
D = 64    # head size
B = 8     # batch == number of cores
P = 128   # SBUF partitions
SBLK = 512         # q-block / s-block width (max fp32 matmul moving dim)
EC = E // P        # 8 contraction chunks
NSB = S // SBLK    # 4 s-blocks
NJT = S // P       # 16 kv tiles

f32 = mybir.dt.float32
f16 = mybir.dt.float16
MMDT = f16          # dtype of all large-matmul operands
MMNP = np.float16   # matching numpy dtype for host-side prep
AF = mybir.ActivationFunctionType

_PATCHED = False


def _patch_tile_drain():
    """The walrus build in this container rejects instructions carrying more
    than one sem wait on the Tile exit Drain. Split the waits across a chain
    of drains, one wait each."""
    global _PATCHED
    if _PATCHED:
        return
    _PATCHED = True

    def _drain_and_barrier(self, tick_clock, wait_clock):
        drain_inst = self.nc.sync.drain()
        wait_clock.add_sem_waits(
            drain_inst.ins, ScopedClock({None: tick_clock.global_clock})
        )
        ins = drain_inst.ins
        si = ins.sync_info
        if si is not None and si.on_wait is not None and len(si.on_wait) > 1:
            waits = list(si.on_wait)
            ins.sync_info = mybir.SyncInfo(
                on_wait=[waits[0]], on_update=list(si.on_update or [])
            )
            for w in waits[1:]:
                d2 = self.nc.sync.drain()
                d2.ins.sync_info = mybir.SyncInfo(on_wait=[w], on_update=[])
        self.nc.all_engine_barrier()
        assert self.sems is not None
        popped = self.nc._tile_sem_poison_stack.pop()
        assert popped is self._sem_poison
        self.nc.clear_and_free_semaphores(list(self.sems.allocated().values()))
        self.nc.all_engine_barrier()

    tile.TileContext._drain_and_barrier = _drain_and_barrier


def _split_multiwaits(nc):
    """This container's walrus rejects instructions carrying more than one
    sem wait (setupSyncWait: 'Too many sync wait commands'). Hoist all but
    the last wait of every instruction onto same-engine NoOps placed
    immediately before it."""
    ctr = 0
    for f in nc.m.functions:
        for bb in f.blocks:
            out = []
            changed = False
            for inst in bb.instructions:
                si = inst.sync_info
                if si is not None and si.on_wait is not None and len(si.on_wait) > 1:
                    waits = list(si.on_wait)
                    for w in waits[:-1]:
                        nop = mybir.InstNoOp(name=f"I-waitsplit-{ctr}")
                        ctr += 1
                        nop.engine = inst.engine
                        nop.sync_info = mybir.SyncInfo(on_wait=[w], on_update=[])
                        out.append(nop)
                    inst.sync_info = mybir.SyncInfo(
                        on_wait=[waits[-1]], on_update=list(si.on_update or [])
                    )
                    changed = True
                out.append(inst)
            if changed:
                bb.instructions = out


def _restructure(nc):
    """Post-build program surgery:
    1. Move the initial input DMAs (weights + x block 0 halves) to the very
       front of the main block so nothing of ours delays their issue (NRT's
       injected preamble still runs first).
    2. Chain x block 1..3 piece DMAs on the previous block's completion
       semaphore per ring, so each ring delivers blocks in order instead of
       round-robin-sharing bandwidth across all queued pieces.
    3. Move the const-pool memsets (Pool engine) from before the entry
       barrier into the tile block.
    4. Delete the main-block entry barrier: cross-engine deps inside the
       tile block are tracked by tile semaphores, and NRT's own preamble
       barrier already synchronizes engine start."""
    f = nc.m.functions[0]
    main, tileb = f.blocks[0], f.blocks[1]
    hoist_ids = {id(i) for i in nc._hoist_dma}
    moved = [i for i in tileb.instructions if id(i) in hoist_ids]
    assert len(moved) == len(nc._hoist_dma), (len(moved), len(nc._hoist_dma))
    rest_tile = [i for i in tileb.instructions if id(i) not in hoist_ids]

    for prev, cur in nc._chain_dma:
        upd = prev.sync_info.on_update
        assert upd, f"chain prev {prev.name} has no completion sem"
        sem = upd[0].id
        w = mybir.SyncWait(
            sync_type="semaphore", id=sem, wait_mode="sem-ge-imm", wait_value=16
        )
        si = cur.sync_info
        waits = list(si.on_wait or []) if si is not None else []
        upds = list(si.on_update or []) if si is not None else []
        cur.sync_info = mybir.SyncInfo(on_wait=waits + [w], on_update=upds)

    memsets = [i for i in main.instructions if isinstance(i, mybir.InstMemset)]
    keep = []
    for i in main.instructions:
        if isinstance(i, mybir.InstMemset):
            continue
        if isinstance(i, mybir.InstDrain):
            continue
        if isinstance(i, mybir.InstEventSemaphore) and i.name.startswith("barrier_"):
            continue
        keep.append(i)
    main.instructions = keep[:1] + moved + keep[1:]
    tileb.instructions = memsets + rest_tile


def _attention(ctx, tc, xt, wqk, wv, y):
    nc = tc.nc
    scale = 1.0 / np.sqrt(D)

    persist = ctx.enter_context(tc.tile_pool(name="persist", bufs=1))
    xpool = ctx.enter_context(tc.tile_pool(name="xts", bufs=1))
    ppool = ctx.enter_context(tc.tile_pool(name="pp", bufs=6))
    rpool = ctx.enter_context(tc.tile_pool(name="rec", bufs=8))
    psproj = ctx.enter_context(tc.tile_pool(name="psproj", bufs=2, space="PSUM"))
    psscore = ctx.enter_context(tc.tile_pool(name="psscore", bufs=2, space="PSUM"))
    pspv = ctx.enter_context(tc.tile_pool(name="pspv", bufs=1, space="PSUM"))
    pstr = ctx.enter_context(tc.tile_pool(name="pstr", bufs=1, space="PSUM"))

    # ---- weights + x stream: both rings, chunk-halves, chained ----------
    EH = EC // 2
    wqk_sb = persist.tile([P, EC, 2 * D], MMDT, tag="wqk")
    wv_sb = persist.tile([P, EC, D], MMDT, tag="wv")
    xt_r = xt.rearrange("p (b c s) -> p b c s", b=NSB, c=EC)
    xts = []
    for b in range(NSB):
        xts_b = xpool.tile([P, EC, SBLK], MMDT, tag=f"xts{b}", name=f"xts_{b}")
        xts.append(xts_b)
    hoist = [
        nc.sync.dma_start(wqk_sb[:], wqk.rearrange("p (c m) -> p c m", c=EC)),
        nc.sync.dma_start(wv_sb[:], wv.rearrange("p (c m) -> p c m", c=EC)),
        nc.sync.dma_start(xts[0][:, :EH], xt_r[:, 0, :EH]),
        nc.scalar.dma_start(xts[0][:, EH:], xt_r[:, 0, EH:]),
    ]
    nc._hoist_dma = [h.ins for h in hoist]
    chain_a = [hoist[2]]
    for b in range(1, NSB):
        chain_a.append(nc.sync.dma_start(xts[b][:, :EH], xt_r[:, b, :EH]))
    chain_b = [hoist[3]]
    chain_b.append(nc.scalar.dma_start(xts[1][:, EH:], xt_r[:, 1, EH:]))

    def _defer_piece(b):
        def emit():
            chain_b.append(nc.scalar.dma_start(xts[b][:, EH:], xt_r[:, b, EH:]))
        return emit

    defer_q = [_defer_piece(2), _defer_piece(3)]
    EORD = [4, 5, 6, 7, 0, 1, 2, 3]  # scalar-ring half lands first

    # ---- PE warm-up: ramp the PE clock while x block 0 streams in --------
    warm_in = persist.tile([P, SBLK], MMDT, tag="warm")
    nc.vector.memset(warm_in[:], 0.25)
    wt = pstr.tile([P, SBLK], f32, tag="tr")
    for _ in range(3):
        nc.tensor.matmul(wt[:], warm_in[:, :P], warm_in[:], start=True, stop=True)

    # ---- constants -------------------------------------------------------
    ident = persist.tile([P, P], f32, tag="ident")
    nc.gpsimd.memset(ident[:], 0.0)
    nc.gpsimd.affine_select(
        out=ident[:], in_=ident[:],
        compare_op=mybir.AluOpType.not_equal, fill=1.0,
        base=0, pattern=[[-1, P]], channel_multiplier=1,
    )
    ident16 = persist.tile([P, P], MMDT, tag="ident16")
    nc.vector.tensor_copy(ident16[:], ident[:])

    # causal step mask: maskW[jj, c] = 1 iff c >= jj + SBLK
    maskW = persist.tile([P, 2 * SBLK], f32, tag="maskw")
    nc.gpsimd.memset(maskW[:], 1.0)
    nc.gpsimd.affine_select(
        out=maskW[:], in_=maskW[:],
        compare_op=mybir.AluOpType.is_ge, fill=0.0,
        base=-SBLK, pattern=[[1, 2 * SBLK]], channel_multiplier=-1,
    )
    mask16 = persist.tile([P, 2 * SBLK], MMDT, tag="mask16")
    nc.vector.tensor_copy(mask16[:], maskW[:])

    # pre-warm the ScalarE activation table (Exp/Ln share one table set) so
    # the one-time ~1.3us ACT_TABLE_LOAD is off the first real exp's path
    actwarm = rpool.tile([D, 4], f32, tag="actwarm")
    nc.scalar.activation(actwarm[:], maskW[:D, :4], AF.Exp)

    # ---- persistent activations -----------------------------------------
    # qk: rows 0:64 = Q^T, rows 64:128 = K^T (straight from packed psum)
    qk = persist.tile([P, S], MMDT, tag="qk")
    # partition-swapped copies: K^T at rows 0:64, Q^T at rows 64:128
    kTlo = persist.tile([D, S], MMDT, tag="ktlo")
    qThi = persist.tile([P, S], MMDT, tag="qthi")  # rows 64:128 used
    vT = persist.tile([D, S], MMDT, tag="vt")
    vAug = persist.tile([P, NJT, 2 * D], MMDT, tag="vaug")
    yT = persist.tile([D, S], f32, tag="ytout")
    ones_f32 = persist.tile([P, NJT, D], f32, tag="ones")
    nc.vector.memset(ones_f32[:], 1.0)
    nc.vector.tensor_copy(vAug[:, :, D:], ones_f32[:])

    def proj(b):
        sl = slice(b * SBLK, (b + 1) * SBLK)
        psQK = psproj.tile([P, SBLK], f32, tag="proj")
        for i, e in enumerate(EORD):
            nc.tensor.matmul(
                psQK[:], wqk_sb[:, e, :], xts[b][:, e, :],
                start=(i == 0), stop=(i == EC - 1),
            )
        nc.vector.tensor_copy(qk[:, sl], psQK[:])
        psV = psproj.tile([P, SBLK], f32, tag="proj")
        for i, e in enumerate(EORD):
            nc.tensor.matmul(
                psV[:D, :], wv_sb[:, e, :], xts[b][:, e, :],
                start=(i == 0), stop=(i == EC - 1),
            )
        # partition swap on the PE: identity matmuls in complementary
        # row/col groups run concurrently; emitted after the V matmuls so
        # the qk copy (DVE) has finished by the time they issue.
        pshift = pstr.tile([P, SBLK], f32, tag="tr")
        nc.tensor.matmul(pshift[:D, :], ident16[D:P, D:P], qk[D:P, sl])
        nc.tensor.matmul(pshift[D:P, :], ident16[:D, :D], qk[:D, sl])
        nc.vector.tensor_copy(kTlo[:, sl], pshift[:D, :])
        nc.vector.tensor_copy(qThi[D:P, sl], pshift[D:P, :])
        nc.vector.tensor_copy(vT[:, sl], psV[:D, :])
        for t in range(4):
            j = 4 * b + t
            psv_t = pstr.tile([P, SBLK], MMDT, tag="tr")
            nc.tensor.transpose(
                psv_t[:, :D], vT[:, j * P : (j + 1) * P], ident16[:D, :D]
            )
            nc.vector.tensor_copy(vAug[:, j, :D], psv_t[:, :D])

    def attn(b, tail_cb=None):
        nj = 4 * b + 4
        psO = pspv.tile([P, SBLK], f32, tag="pv")
        pairs = [(jp, jp + 1) for jp in range(0, nj, 2)]

        def scores_pair(pi):
            j0, j1 = pairs[pi]
            ps = psscore.tile([P, 2 * SBLK], f32, tag="score")
            # narrow only the strongly-masked tiles (t>=2); the (0,1) pair
            # stays full-width so one exp can cover both banks contiguously
            o0 = max(0, (j0 - 4 * b) * P)
            o1 = max(0, (j1 - 4 * b) * P)
            o0 = o0 if o0 >= 2 * P else 0
            o1 = o1 if o1 >= 2 * P else 0
            q0 = slice(b * SBLK + o0, (b + 1) * SBLK)
            q1 = slice(b * SBLK + o1, (b + 1) * SBLK)
            # two PE row-groups: rows 0:64 (kTlo/qk) and 64:128 (qk/qThi)
            nc.tensor.matmul(
                ps[:, o0:SBLK], kTlo[:, j0 * P : (j0 + 1) * P], qk[:D, q0],
            )
            nc.tensor.matmul(
                ps[:, SBLK + o1 :], qk[D:P, j1 * P : (j1 + 1) * P], qThi[D:P, q1],
            )
            return (j0, j1, ps)

        inflight = scores_pair(0)
        for pi in range(len(pairs)):
            j0, j1, ps = inflight
            pt = ppool.tile([P, 2 * SBLK], MMDT, tag="pt")
            offs = [max(0, (j - 4 * b) * P) for j in (j0, j1)]
            eoffs = [o if o >= 2 * P else 0 for o in offs]
            if eoffs == [0, 0]:
                # (nearly) fully-visible pair: one batched exp over both banks
                nc.scalar.activation(pt[:], ps[:], AF.Exp, scale=float(scale))
            else:
                # strongly-masked pair: exp only the causally-reachable columns
                for k, off in enumerate(eoffs):
                    nc.scalar.activation(
                        pt[:, k * SBLK + off : (k + 1) * SBLK],
                        ps[:, k * SBLK + off : (k + 1) * SBLK],
                        AF.Exp, scale=float(scale),
                    )
            if pi == 0 and defer_q:
                defer_q.pop(0)()
            for k, j in enumerate((j0, j1)):
                t = j - 4 * b
                if t >= 0:
                    off = eoffs[k]
                    nc.vector.tensor_mul(
                        pt[:, k * SBLK + off : (k + 1) * SBLK],
                        pt[:, k * SBLK + off : (k + 1) * SBLK],
                        mask16[:, SBLK - t * P + off : 2 * SBLK - t * P],
                    )
            if pi + 1 < len(pairs):
                inflight = scores_pair(pi + 1)
            if tail_cb is not None and pi == len(pairs) - 1:
                # psO columns untouched by this last pair are already final:
                # normalize + store them while this pair's PV runs
                tail_cb(psO)
            for k, j in enumerate((j0, j1)):
                off = eoffs[k]
                nc.tensor.matmul(
                    psO[:, off:], vAug[:, j, :],
                    pt[:, k * SBLK + off : (k + 1) * SBLK],
                    start=(j == 0), stop=(j == nj - 1),
                )
        return psO

    def out_part(b, psO, c0, c1):
        # rows 64:128 of psO hold the softmax denominator, pre-broadcast.
        # 1/s as exp(-ln s) on ScalarE: same ACT table as the softmax exp,
        # ~6x faster than the DVE reciprocal at this shape.
        sl = slice(b * SBLK + c0, b * SBLK + c1)
        lns = rpool.tile([D, SBLK], f32, tag="lns")
        nc.scalar.activation(lns[:, c0:c1], psO[D:P, c0:c1], AF.Ln)
        rcp = rpool.tile([D, SBLK], f32, tag="rcp")
        nc.scalar.activation(rcp[:, c0:c1], lns[:, c0:c1], AF.Exp, scale=-1.0)
        nc.vector.tensor_mul(yT[:, sl], psO[:D, c0:c1], rcp[:, c0:c1])
        eng = nc.scalar if (b == NSB - 1 and c0 > 0) else nc.sync
        eng.dma_start(y[:, sl], yT[:, sl])

    # schedule: keep independent proj work ahead of each attn phase; the
    # last attn phase (exp-latency-bound) overlaps its own output tail
    proj(0)
    proj(1)
    o0 = attn(0)
    out_part(0, o0, 0, SBLK)
    proj(2)
    o1 = attn(1)
    out_part(1, o1, 0, SBLK)
    proj(3)
    o2 = attn(2)
    out_part(2, o2, 0, SBLK)
    half = SBLK // 2
    o3 = attn(3, tail_cb=lambda psO: out_part(3, psO, 0, half))
    out_part(3, o3, half, SBLK)
    nc._chain_dma = [
        (c[i].ins, c[i + 1].ins)
        for c in (chain_a, chain_b)
        for i in range(len(c) - 1)
    ]


def build_nc():
    from contextlib import ExitStack

    _patch_tile_drain()
    nc = bass.Bass(target_bir_lowering=False, enable_partition_id=False)
    xt = nc.dram_tensor("xt", [P, NSB * EC * SBLK], MMDT, kind="ExternalInput")
    wqk = nc.dram_tensor("wqk", [P, EC * 2 * D], MMDT, kind="ExternalInput")
    wv = nc.dram_tensor("wv", [P, EC * D], MMDT, kind="ExternalInput")
    y = nc.dram_tensor("y", [D, S], f32, kind="ExternalOutput")
    with tile.TileContext(nc) as tc:
        with ExitStack() as ctx:
            _attention(ctx, tc, xt, wqk, wv, y)
    _restructure(nc)
    return nc


def make_in_maps(x, Wq, Wk, Wv):
    # weights pre-tiled to [P, EC, cols]: row (c p) of W -> [p][c]
    wqk_cat = np.concatenate([Wq, Wk], axis=1).astype(MMNP)  # [E, 2D]
    wqk_arr = np.ascontiguousarray(
        wqk_cat.reshape(EC, P, 2 * D).transpose(1, 0, 2).reshape(P, EC * 2 * D)
    )
    wv_arr = np.ascontiguousarray(
        np.asarray(Wv).astype(MMNP).reshape(EC, P, D).transpose(1, 0, 2).reshape(P, EC * D)
    )
    x = np.asarray(x)
    maps = []
    for b in range(B):
        xt = x[b].T.astype(MMNP)  # [E, S]
        # [P, NSB, EC, SBLK]: xa[p, blk, c, s] = xt[c*128+p, blk*512+s]
        xa = xt.reshape(EC, P, NSB, SBLK).transpose(1, 2, 0, 3)
        maps.append(
            {
                "xt": np.ascontiguousarray(xa.reshape(P, NSB * EC * SBLK)),
                "wqk": wqk_arr,
                "wv": wv_arr,
            }
        )
    return maps


_NC = None


def kernel(x, Wq, Wk, Wv, _trace=False, _tmpdir=None):
    from concourse.bass_utils import run_bass_kernel_spmd

    global _NC
    if _NC is None:
        _NC = build_nc()
        _split_multiwaits(_NC)  # walrus-only legalization; breaks CoreSim
    in_maps = make_in_maps(x, Wq, Wk, Wv)
    res = run_bass_kernel_spmd(
        _NC, in_maps, core_ids=list(range(B)), trace=_trace, tmpdir=_tmpdir
    )
    out = np.ascontiguousarray(
        np.stack([r["y"].T for r in res.results], axis=0), dtype=np.float32
    )
    if _trace:
        kernel.last_results = res
    return out


# revision 13
# speedup vs baseline: 1.3015x; 1.0105x over previous
"""Single-head causal attention on 8 trn2 NeuronCores (one batch element per core).

Problem: x [8, 2048, 1024], Wq/Wk/Wv [1024, 64] -> out [8, 2048, 64]
  q = x@Wq; k = x@Wk; v = x@Wv; out = causal_softmax(q k^T / sqrt(64)) @ v

Strategy (per core, batch-parallel across the 8 cores):
  - Host pre-transposes + pre-tiles x to [P, NSB, EC, SBLK] fp16 so every DMA
    piece is 4KB-contiguous per partition and the QKV projections contract
    over E with E on SBUF partitions.
  - x streams on BOTH HWDGE rings (each ring caps at ~180 GB/s; the 16 DMA
    engines are split between rings), chunk-halves per ring, with each
    block's piece chained on the previous block's completion semaphore —
    rings process queued transfers concurrently, so without the chain every
    piece finishes late together. The first pieces are placed ahead of the
    (NRT-injected) entry barriers; NRT's preamble still gates issue at
    ~6.8us, but nothing of ours adds to it.
  - Q^T/K^T packed as one [Wq|Wk] matmul; partition-swapped copies (K^T at
    rows 0:64, Q^T at rows 64:128) via PE identity-shift matmuls in
    complementary row/col groups (concurrent, ~0.2us/block).
  - Scores transposed (P^T[kv, q]) two tiles at a time in disjoint PE row
    groups; softmax normalization rides the PV matmul via a ones column in V
    (row 64 of PV output = denominator). No max-subtraction needed.
  - exp on ScalarE straight out of PSUM with the 1/sqrt(D) scale folded in;
    ACT table pre-warmed at tile entry. Softmax divide as exp(-ln s) on
    ScalarE (the DVE reciprocal is ~6x slower at this shape).
  - Final block's output is produced in two column halves: columns 0:256 of
    the PV accumulator are final one pair early, so the first half's
    normalize + store overlaps the last pair's exp/PV.
  - Block schedule proj0,proj1,attn0,proj2,attn1,proj3,attn2,attn3 keeps
    independent projection work between attention phases.
"""

import numpy as np

import concourse.bass as bass
import concourse.mybir as mybir
import concourse.tile as tile
from concourse.vector_clock import ScopedClock

S = 2048  # sequence length
E = 1024  # embed dim
D = 64    # head size
B = 8     # batch == number of cores
P = 128   # SBUF partitions
SBLK = 512         # q-block / s-block width (max fp32 matmul moving dim)
EC = E // P        # 8 contraction chunks
NSB = S // SBLK    # 4 s-blocks
NJT = S // P       # 16 kv tiles

f32 = mybir.dt.float32
f16 = mybir.dt.float16
MMDT = f16          # dtype of all large-matmul operands
MMNP = np.float16   # matching numpy dtype for host-side prep
AF = mybir.ActivationFunctionType

_PATCHED = False


def _patch_tile_drain():
    """The walrus build in this container rejects instructions carrying more
    than one sem wait on the Tile exit Drain. Split the waits across a chain
    of drains, one wait each."""
    global _PATCHED
    if _PATCHED:
        return
    _PATCHED = True

    def _drain_and_barrier(self, tick_clock, wait_clock):
        drain_inst = self.nc.sync.drain()
        wait_clock.add_sem_waits(
            drain_inst.ins, ScopedClock({None: tick_clock.global_clock})
        )
        ins = drain_inst.ins
        si = ins.sync_info
        if si is not None and si.on_wait is not None and len(si.on_wait) > 1:
            waits = list(si.on_wait)
            ins.sync_info = mybir.SyncInfo(
                on_wait=[waits[0]], on_update=list(si.on_update or [])
            )
            for w in waits[1:]:
                d2 = self.nc.sync.drain()
                d2.ins.sync_info = mybir.SyncInfo(on_wait=[w], on_update=[])
        self.nc.all_engine_barrier()
        assert self.sems is not None
        popped = self.nc._tile_sem_poison_stack.pop()
        assert popped is self._sem_poison
        self.nc.clear_and_free_semaphores(list(self.sems.allocated().values()))
        self.nc.all_engine_barrier()

    tile.TileContext._drain_and_barrier = _drain_and_barrier


def _split_multiwaits(nc):
    """This container's walrus rejects instructions carrying more than one
    sem wait (setupSyncWait: 'Too many sync wait commands'). Hoist all but
    the last wait of every instruction onto same-engine NoOps placed
    immediately before it."""
    ctr = 0
    for f in nc.m.functions:
        for bb in f.blocks:
            out = []
            changed = False
            for inst in bb.instructions:
                si = inst.sync_info
                if si is not None and si.on_wait is not None and len(si.on_wait) > 1:
                    waits = list(si.on_wait)
                    for w in waits[:-1]:
                        nop = mybir.InstNoOp(name=f"I-waitsplit-{ctr}")
                        ctr += 1
                        nop.engine = inst.engine
                        nop.sync_info = mybir.SyncInfo(on_wait=[w], on_update=[])
                        out.append(nop)
                    inst.sync_info = mybir.SyncInfo(
                        on_wait=[waits[-1]], on_update=list(si.on_update or [])
                    )
                    changed = True
                out.append(inst)
            if changed:
                bb.instructions = out


def _restructure(nc):
    """Post-build program surgery:
    1. Move the initial input DMAs (weights + x block 0 halves) to the very
       front of the main block so nothing of ours delays their issue (NRT's
       injected preamble still runs first).
    2. Chain x block 1..3 piece DMAs on the previous block's completion
       semaphore per ring, so each ring delivers blocks in order instead of
       round-robin-sharing bandwidth across all queued pieces.
    3. Move the const-pool memsets (Pool engine) from before the entry
       barrier into the tile block.
    4. Delete the main-block entry barrier: cross-engine deps inside the
       tile block are tracked by tile semaphores, and NRT's own preamble
       barrier already synchronizes engine start."""
    f = nc.m.functions[0]
    main, tileb = f.blocks[0], f.blocks[1]
    hoist_ids = {id(i) for i in nc._hoist_dma}
    moved = [i for i in tileb.instructions if id(i) in hoist_ids]
    assert len(moved) == len(nc._hoist_dma), (len(moved), len(nc._hoist_dma))
    rest_tile = [i for i in tileb.instructions if id(i) not in hoist_ids]

    for prev, cur in nc._chain_dma:
        upd = prev.sync_info.on_update
        assert upd, f"chain prev {prev.name} has no completion sem"
        sem = upd[0].id
        w = mybir.SyncWait(
            sync_type="semaphore", id=sem, wait_mode="sem-ge-imm", wait_value=16
        )
        si = cur.sync_info
        waits = list(si.on_wait or []) if si is not None else []
        upds = list(si.on_update or []) if si is not None else []
        cur.sync_info = mybir.SyncInfo(on_wait=waits + [w], on_update=upds)

    memsets = [i for i in main.instructions if isinstance(i, mybir.InstMemset)]
    keep = []
    for i in main.instructions:
        if isinstance(i, mybir.InstMemset):
            continue
        if isinstance(i, mybir.InstDrain):
            continue
        if isinstance(i, mybir.InstEventSemaphore) and i.name.startswith("barrier_"):
            continue
        keep.append(i)
    main.instructions = keep[:1] + moved + keep[1:]
    tileb.instructions = memsets + rest_tile


def _attention(ctx, tc, xt, wqk, wv, y):
    nc = tc.nc
    scale = 1.0 / np.sqrt(D)

    persist = ctx.enter_context(tc.tile_pool(name="persist", bufs=1))
    xpool = ctx.enter_context(tc.tile_pool(name="xts", bufs=1))
    ppool = ctx.enter_context(tc.tile_pool(name="pp", bufs=6))
    rpool = ctx.enter_context(tc.tile_pool(name="rec", bufs=8))
    psproj = ctx.enter_context(tc.tile_pool(name="psproj", bufs=2, space="PSUM"))
    psscore = ctx.enter_context(tc.tile_pool(name="psscore", bufs=2, space="PSUM"))
    pspv = ctx.enter_context(tc.tile_pool(name="pspv", bufs=1, space="PSUM"))
    pstr = ctx.enter_context(tc.tile_pool(name="pstr", bufs=1, space="PSUM"))

    # ---- weights + x stream: both rings, chunk-halves, chained ----------
    EH = EC // 2
    wqk_sb = persist.tile([P, EC, 2 * D], MMDT, tag="wqk")
    wv_sb = persist.tile([P, EC, D], MMDT, tag="wv")
    xt_r = xt.rearrange("p (b c s) -> p b c s", b=NSB, c=EC)
    xts = []
    for b in range(NSB):
        xts_b = xpool.tile([P, EC, SBLK], MMDT, tag=f"xts{b}", name=f"xts_{b}")
        xts.append(xts_b)
    hoist = [
        nc.sync.dma_start(wqk_sb[:], wqk.rearrange("p (c m) -> p c m", c=EC)),
        nc.sync.dma_start(wv_sb[:], wv.rearrange("p (c m) -> p c m", c=EC)),
        nc.sync.dma_start(xts[0][:, :EH], xt_r[:, 0, :EH]),
        nc.scalar.dma_start(xts[0][:, EH:], xt_r[:, 0, EH:]),
    ]
    nc._hoist_dma = [h.ins for h in hoist]
    chain_a = [hoist[2]]
    for b in range(1, NSB):
        chain_a.append(nc.sync.dma_start(xts[b][:, :EH], xt_r[:, b, :EH]))
    chain_b = [hoist[3]]
    chain_b.append(nc.scalar.dma_start(xts[1][:, EH:], xt_r[:, 1, EH:]))

    def _defer_piece(b):
        def emit():
            chain_b.append(nc.scalar.dma_start(xts[b][:, EH:], xt_r[:, b, EH:]))
        return emit

    defer_q = [_defer_piece(2), _defer_piece(3)]
    EORD = [4, 5, 6, 7, 0, 1, 2, 3]  # scalar-ring half lands first

    # ---- PE warm-up: ramp the PE clock while x block 0 streams in --------
    warm_in = persist.tile([P, SBLK], MMDT, tag="warm")
    nc.vector.memset(warm_in[:], 0.25)
    wt = pstr.tile([P, SBLK], f32, tag="tr")
    for _ in range(3):
        nc.tensor.matmul(wt[:], warm_in[:, :P], warm_in[:], start=True, stop=True)

    # ---- constants -------------------------------------------------------
    ident = persist.tile([P, P], f32, tag="ident")
    nc.gpsimd.memset(ident[:], 0.0)
    nc.gpsimd.affine_select(
        out=ident[:], in_=ident[:],
        compare_op=mybir.AluOpType.not_equal, fill=1.0,
        base=0, pattern=[[-1, P]], channel_multiplier=1,
    )
    ident16 = persist.tile([P, P], MMDT, tag="ident16")
    nc.vector.tensor_copy(ident16[:], ident[:])

    # causal step mask: maskW[jj, c] = 1 iff c >= jj + SBLK
    maskW = persist.tile([P, 2 * SBLK], f32, tag="maskw")
    nc.gpsimd.memset(maskW[:], 1.0)
    nc.gpsimd.affine_select(
        out=maskW[:], in_=maskW[:],
        compare_op=mybir.AluOpType.is_ge, fill=0.0,
        base=-SBLK, pattern=[[1, 2 * SBLK]], channel_multiplier=-1,
    )
    mask16 = persist.tile([P, 2 * SBLK], MMDT, tag="mask16")
    nc.vector.tensor_copy(mask16[:], maskW[:])

    # pre-warm the ScalarE activation table (Exp/Ln share one table set) so
    # the one-time ~1.3us ACT_TABLE_LOAD is off the first real exp's path
    actwarm = rpool.tile([D, 4], f32, tag="actwarm")
    nc.scalar.activation(actwarm[:], maskW[:D, :4], AF.Exp)

    # ---- persistent activations -----------------------------------------
    # qk: rows 0:64 = Q^T, rows 64:128 = K^T (straight from packed psum)
    qk = persist.tile([P, S], MMDT, tag="qk")
    # partition-swapped copies: K^T at rows 0:64, Q^T at rows 64:128
    kTlo = persist.tile([D, S], MMDT, tag="ktlo")
    qThi = persist.tile([P, S], MMDT, tag="qthi")  # rows 64:128 used
    vT = persist.tile([D, S], MMDT, tag="vt")
    vAug = persist.tile([P, NJT, 2 * D], MMDT, tag="vaug")
    yT = persist.tile([D, S], f32, tag="ytout")
    ones_f32 = persist.tile([P, NJT, D], f32, tag="ones")
    nc.vector.memset(ones_f32[:], 1.0)
    nc.vector.tensor_copy(vAug[:, :, D:], ones_f32[:])

    def proj(b):
        sl = slice(b * SBLK, (b + 1) * SBLK)
        psQK = psproj.tile([P, SBLK], f32, tag="proj")
        for i, e in enumerate(EORD):
            nc.tensor.matmul(
                psQK[:], wqk_sb[:, e, :], xts[b][:, e, :],
                start=(i == 0), stop=(i == EC - 1),
            )
        nc.vector.tensor_copy(qk[:, sl], psQK[:])
        psV = psproj.tile([P, SBLK], f32, tag="proj")
        for i, e in enumerate(EORD):
            nc.tensor.matmul(
                psV[:D, :], wv_sb[:, e, :], xts[b][:, e, :],
                start=(i == 0), stop=(i == EC - 1),
            )
        # partition swap on the PE: identity matmuls in complementary
        # row/col groups run concurrently; emitted after the V matmuls so
        # the qk copy (DVE) has finished by the time they issue.
        pshift = pstr.tile([P, SBLK], f32, tag="tr")
        nc.tensor.matmul(pshift[:D, :], ident16[D:P, D:P], qk[D:P, sl])
        nc.tensor.matmul(pshift[D:P, :], ident16[:D, :D], qk[:D, sl])
        nc.vector.tensor_copy(kTlo[:, sl], pshift[:D, :])
        nc.vector.tensor_copy(qThi[D:P, sl], pshift[D:P, :])
        nc.vector.tensor_copy(vT[:, sl], psV[:D, :])
        for t in range(4):
            j = 4 * b + t
            psv_t = pstr.tile([P, SBLK], MMDT, tag="tr")
            nc.tensor.transpose(
                psv_t[:, :D], vT[:, j * P : (j + 1) * P], ident16[:D, :D]
            )
            nc.vector.tensor_copy(vAug[:, j, :D], psv_t[:, :D])

    def attn(b, tail_cb=None):
        nj = 4 * b + 4
        psO = pspv.tile([P, SBLK], f32, tag="pv")
        pairs = [(jp, jp + 1) for jp in range(0, nj, 2)]

        def scores_pair(pi):
            j0, j1 = pairs[pi]
            ps = psscore.tile([P, 2 * SBLK], f32, tag="score")
            # narrow only the strongly-masked tiles (t>=2); the (0,1) pair
            # stays full-width so one exp can cover both banks contiguously
            o0 = max(0, (j0 - 4 * b) * P)
            o1 = max(0, (j1 - 4 * b) * P)
            o0 = o0 if o0 >= 2 * P else 0
            o1 = o1 if o1 >= 2 * P else 0
            q0 = slice(b * SBLK + o0, (b + 1) * SBLK)
            q1 = slice(b * SBLK + o1, (b + 1) * SBLK)
            # two PE row-groups: rows 0:64 (kTlo/qk) and 64:128 (qk/qThi)
            nc.tensor.matmul(
                ps[:, o0:SBLK], kTlo[:, j0 * P : (j0 + 1) * P], qk[:D, q0],
            )
            nc.tensor.matmul(
                ps[:, SBLK + o1 :], qk[D:P, j1 * P : (j1 + 1) * P], qThi[D:P, q1],
            )
            return (j0, j1, ps)

        inflight = scores_pair(0)
        for pi in range(len(pairs)):
            j0, j1, ps = inflight
            pt = ppool.tile([P, 2 * SBLK], MMDT, tag="pt")
            offs = [max(0, (j - 4 * b) * P) for j in (j0, j1)]
            eoffs = [o if o >= 2 * P else 0 for o in offs]
            if eoffs == [0, 0]:
                # (nearly) fully-visible pair: one batched exp over both banks
                nc.scalar.activation(pt[:], ps[:], AF.Exp, scale=float(scale))
            else:
                # strongly-masked pair: exp only the causally-reachable columns
                for k, off in enumerate(eoffs):
                    nc.scalar.activation(
                        pt[:, k * SBLK + off : (k + 1) * SBLK],
                        ps[:, k * SBLK + off : (k + 1) * SBLK],
                        AF.Exp, scale=float(scale),
                    )
            if pi == 0 and defer_q:
                defer_q.pop(0)()
            for k, j in enumerate((j0, j1)):
                t = j - 4 * b
                if t >= 0:
                    off = eoffs[k]
                    nc.vector.tensor_mul(
                        pt[:, k * SBLK + off : (k + 1) * SBLK],
                        pt[:, k * SBLK + off : (k + 1) * SBLK],
                        mask16[:, SBLK - t * P + off : 2 * SBLK - t * P],
                    )
            if pi + 1 < len(pairs):
                inflight = scores_pair(pi + 1)
            if tail_cb is not None and pi == len(pairs) - 1:
                # psO columns untouched by this last pair are already final:
                # normalize + store them while this pair's PV runs
                tail_cb(psO)
            for k, j in enumerate((j0, j1)):
                off = eoffs[k]
                nc.tensor.matmul(
                    psO[:, off:], vAug[:, j, :],
                    pt[:, k * SBLK + off : (k + 1) * SBLK],
                    start=(j == 0), stop=(j == nj - 1),
                )
        return psO

    def out_part(b, psO, c0, c1):
        # rows 64:128 of psO hold the softmax denominator, pre-broadcast.
        # 1/s as exp(-ln s) on ScalarE: same ACT table as the softmax exp,
        # ~6x faster than the DVE reciprocal at this shape.
        sl = slice(b * SBLK + c0, b * SBLK + c1)
        lns = rpool.tile([D, SBLK], f32, tag="lns")
        nc.scalar.activation(lns[:, c0:c1], psO[D:P, c0:c1], AF.Ln)
        rcp = rpool.tile([D, SBLK], f32, tag="rcp")
        nc.scalar.activation(rcp[:, c0:c1], lns[:, c0:c1], AF.Exp, scale=-1.0)
        nc.vector.tensor_mul(yT[:, sl], psO[:D, c0:c1], rcp[:, c0:c1])
        eng = nc.scalar if (b == NSB - 1 and c0 > 0) else nc.sync
        eng.dma_start(y[:, sl], yT[:, sl])

    # schedule: attention immediately after its projection so the ScalarE
    # exp stream starts as early as possible; the last attn phase
    # (exp-latency-bound) overlaps its own output tail
    proj(0)
    o0 = attn(0)
    out_part(0, o0, 0, SBLK)
    proj(1)
    o1 = attn(1)
    out_part(1, o1, 0, SBLK)
    proj(2)
    o2 = attn(2)
    out_part(2, o2, 0, SBLK)
    proj(3)
    half = SBLK // 2
    o3 = attn(3, tail_cb=lambda psO: out_part(3, psO, 0, half))
    out_part(3, o3, half, SBLK)
    nc._chain_dma = [
        (c[i].ins, c[i + 1].ins)
        for c in (chain_a, chain_b)
        for i in range(len(c) - 1)
    ]


def build_nc():
    from contextlib import ExitStack

    _patch_tile_drain()
    nc = bass.Bass(target_bir_lowering=False, enable_partition_id=False)
    xt = nc.dram_tensor("xt", [P, NSB * EC * SBLK], MMDT, kind="ExternalInput")
    wqk = nc.dram_tensor("wqk", [P, EC * 2 * D], MMDT, kind="ExternalInput")
    wv = nc.dram_tensor("wv", [P, EC * D], MMDT, kind="ExternalInput")
    y = nc.dram_tensor("y", [D, S], f32, kind="ExternalOutput")
    with tile.TileContext(nc) as tc:
        with ExitStack() as ctx:
            _attention(ctx, tc, xt, wqk, wv, y)
    _restructure(nc)
    return nc


def make_in_maps(x, Wq, Wk, Wv):
    # weights pre-tiled to [P, EC, cols]: row (c p) of W -> [p][c]
    wqk_cat = np.concatenate([Wq, Wk], axis=1).astype(MMNP)  # [E, 2D]
    wqk_arr = np.ascontiguousarray(
        wqk_cat.reshape(EC, P, 2 * D).transpose(1, 0, 2).reshape(P, EC * 2 * D)
    )
    wv_arr = np.ascontiguousarray(
        np.asarray(Wv).astype(MMNP).reshape(EC, P, D).transpose(1, 0, 2).reshape(P, EC * D)
    )
    x = np.asarray(x)
    maps = []
    for b in range(B):
        xt = x[b].T.astype(MMNP)  # [E, S]
        # [P, NSB, EC, SBLK]: xa[p, blk, c, s] = xt[c*128+p, blk*512+s]
        xa = xt.reshape(EC, P, NSB, SBLK).transpose(1, 2, 0, 3)
        maps.append(
            {
                "xt": np.ascontiguousarray(xa.reshape(P, NSB * EC * SBLK)),
                "wqk": wqk_arr,
                "wv": wv_arr,
            }
        )
    return maps


_NC = None


def kernel(x, Wq, Wk, Wv, _trace=False, _tmpdir=None):
    from concourse.bass_utils import run_bass_kernel_spmd

    global _NC
    if _NC is None:
        _NC = build_nc()
        _split_multiwaits(_NC)  # walrus-only legalization; breaks CoreSim
    in_maps = make_in_maps(x, Wq, Wk, Wv)
    res = run_bass_kernel_spmd(
        _NC, in_maps, core_ids=list(range(B)), trace=_trace, tmpdir=_tmpdir
    )
    out = np.ascontiguousarray(
        np.stack([r["y"].T for r in res.results], axis=0), dtype=np.float32
    )
    if _trace:
        kernel.last_results = res
    return out
